# revision 5
# baseline (speedup 1.0000x reference)
"""Trainium2 Bass kernel for nn_DecoderSACA (single-step decoder w/ self+cross
attention, LSTM cell, and vocab projection).

Sharding: data-parallel over batch B=64 across 8 NeuronCores (8 batches/core)
for everything through the LSTM; the final fc projection is tensor-parallel
over the (padded) vocab after an on-device AllGather of h_new.

Device layout convention: activations are stored transposed (feature on the
SBUF partition dim, batch/token on the free dim) so every matmul contracts
along partitions naturally.  All biases are folded in with a ones-row trick.
Attention scores for all 16 heads of one batch come from a single matmul
chain with a block-diagonal Q operand (built on device with a constant mask).
Matmul inputs use the float32r dtype (TF32-like, full PE rate at N=512).
"""

import numpy as np

# ---- problem constants ----
B, S, H, NH, V = 64, 2048, 1024, 16, 50257
MAXLEN = 1024
HD = H // NH           # 64
NC_N = 8               # cores
BPC = B // NC_N        # 8 batches per core
CK = 512               # token chunk size
KT = H // 128          # 8 feature ktiles
# vocab shard: pad V so shards are equal and even (fp32r matmuls need an
# even moving-operand innermost count); each core handles VS columns
VS = (V + NC_N - 1) // NC_N
VS += VS % 2                            # 6284
VPAD = VS * NC_N                        # 50272


def _full_cfg(step):
    return dict(S=S, STEP=int(step), VS=VS, H=H, NH=NH, BPC=BPC, CK=CK)


_BUILD_CACHE = {}


def _build(cfg):
    import concourse.bass as bass
    import concourse.tile as tile
    from concourse import bacc, mybir, masks

    f32, f32r = mybir.dt.float32, mybir.dt.float32r
    AFT = mybir.ActivationFunctionType
    AX = mybir.AxisListType

    Hc, NHc, BPCc, CKc = cfg["H"], cfg["NH"], cfg["BPC"], cfg["CK"]
    Sc, STEPc, VSc = cfg["S"], cfg["STEP"], cfg["VS"]
    KTc = Hc // 128
    HDc = Hc // NHc
    G4 = 4 * Hc                       # gates width
    ZK = 3 * Hc                       # z contraction (emb + ca + h), + ones row
    assert CKc % 128 == 0 and Hc % 512 == 0 and Sc % CKc == 0 and STEPc % CKc == 0

    nc = bacc.Bacc("TRN2", target_bir_lowering=False, debug=False, num_devices=NC_N)

    def din(name, shape, dt=f32r):
        return nc.dram_tensor(name, list(shape), dt, kind="ExternalInput").ap()

    def dout(name, shape, dt=f32):
        return nc.dram_tensor(name, list(shape), dt, kind="ExternalOutput").ap()

    enc_T = din("enc_T", (BPCc, Hc, Sc))
    ctx_T = din("ctx_T", (BPCc, Hc, STEPc))
    emb_T = din("emb_T", (Hc, BPCc))
    h_T = din("h_T", (Hc, BPCc))
    c_nat = din("c_nat", (BPCc, Hc), f32)
    w_attn = {}
    for nm in ("qsa", "ksa", "vsa", "qca", "kca", "vca"):
        w_attn[nm] = din(f"w_{nm}", (Hc + 1, Hc))
    wc = din("wc", (ZK + 1, G4))
    fcw = din("fcw", (Hc + 1, VSc))
    qmask = din("qmask", (Hc, NHc), f32)
    ones_in = din("ones_in", (1, 512))

    logits = dout("logits", (B, VSc))
    h_out = dout("h_out", (BPCc, Hc))
    c_out = dout("c_out", (BPCc, Hc))

    OCH = Hc // 512                   # output-feature 512-chunks (2)

    with tile.TileContext(nc) as tc:
        import contextlib
        est = contextlib.ExitStack()
        with est:
            persist = est.enter_context(tc.tile_pool(name="persist", bufs=1))
            ps_proj = est.enter_context(tc.tile_pool(name="ps_proj", bufs=4, space="PSUM"))
            ps_sc = est.enter_context(tc.tile_pool(name="ps_sc", bufs=2, space="PSUM"))
            ps_av = est.enter_context(tc.tile_pool(name="ps_av", bufs=1, space="PSUM"))
            dpool = est.enter_context(tc.tile_pool(name="dram", bufs=1, space="DRAM"))

            ident = persist.tile([128, 128], f32, name="ident")
            masks.make_identity(nc, ident[:])
            ones = persist.tile([1, 512], f32r, name="ones")
            nc.sync.dma_start(ones[:], ones_in[:])
            qm = []
            for k in range(KTc):
                t = persist.tile([128, NHc], f32, name=f"qm{k}", tag=f"qm{k}")
                nc.sync.dma_start(t[:], qmask[k * 128:(k + 1) * 128, :])
                qm.append(t)
            emb_t, h_t = [], []
            for k in range(KTc):
                t = persist.tile([128, BPCc], f32r, name=f"emb{k}", tag=f"emb{k}")
                nc.sync.dma_start(t[:], emb_T[k * 128:(k + 1) * 128, :])
                emb_t.append(t)
                t = persist.tile([128, BPCc], f32r, name=f"ht{k}", tag=f"ht{k}")
                nc.sync.dma_start(t[:], h_T[k * 128:(k + 1) * 128, :])
                h_t.append(t)
            c_t = persist.tile([BPCc, Hc], f32, name="c_t")
            nc.sync.dma_start(c_t[:], c_nat[:])
            sa_T = [persist.tile([128, BPCc], f32r, name=f"saT{k}", tag=f"saT{k}")
                    for k in range(KTc)]
            ca_T = [persist.tile([128, BPCc], f32r, name=f"caT{k}", tag=f"caT{k}")
                    for k in range(KTc)]

            def attention(tag, q_src, kv_dram, L, wq_t, wq_b, wk_t, wk_b, wv_t, wv_b,
                          out_T, apool, spool):
                """q_src: KT sbuf tiles (128, BPC) f32r.  kv_dram: (BPC, H, L) dram.
                Writes out_T: KT tiles (128, BPC) f32r (features x batch)."""
                nch = L // CKc
                # ---- Q projection + block-diagonal Q ----
                qT = []
                for m in range(KTc):
                    ps = ps_sc.tile([128, BPCc], f32, name=f"{tag}_qps{m}", tag="sc")
                    for k in range(KTc):
                        nc.tensor.matmul(ps[:], wq_t[k][:, m * 128:(m + 1) * 128],
                                         q_src[k][:], start=(k == 0), stop=False)
                    nc.tensor.matmul(ps[:], wq_b[:, m * 128:(m + 1) * 128],
                                     ones[:, :BPCc], start=False, stop=True)
                    t = apool.tile([128, BPCc], f32, name=f"{tag}_qT{m}", tag=f"{tag}_qT{m}")
                    nc.vector.tensor_copy(t[:], ps[:])
                    qT.append(t)
                qbd = {}
                for b in range(BPCc):
                    for k in range(KTc):
                        t = apool.tile([128, NHc], f32r, name=f"{tag}_qbd{b}_{k}",
                                       tag=f"{tag}_qbd{b}_{k}")
                        nc.vector.tensor_scalar_mul(t[:], qm[k][:], qT[k][:, b:b + 1])
                        qbd[(b, k)] = t
                # ---- chunked K/V projection + attention ----
                for b in range(BPCc):
                    sums = apool.tile([NHc, nch], f32, name=f"{tag}_sums{b}", tag="sums")
                    avp = ps_av.tile([NHc, Hc], f32, name=f"{tag}_avp{b}", tag="av")
                    for lc in range(nch):
                        col0 = lc * CKc
                        kv = []
                        for k in range(KTc):
                            t = spool.tile([128, CKc], f32r, name=f"{tag}_kv{k}",
                                           tag=f"kv{k}")
                            nc.sync.dma_start(
                                t[:], kv_dram[b, k * 128:(k + 1) * 128, col0:col0 + CKc])
                            kv.append(t)
                        # K^T chunk: (features, CK)
                        kT = []
                        for m in range(KTc):
                            ps = ps_proj.tile([128, CKc], f32, name=f"{tag}_kps{m}",
                                              tag="proj")
                            for k in range(KTc):
                                nc.tensor.matmul(ps[:], wk_t[k][:, m * 128:(m + 1) * 128],
                                                 kv[k][:], start=(k == 0), stop=False)
                            nc.tensor.matmul(ps[:], wk_b[:, m * 128:(m + 1) * 128],
                                             ones[:, :CKc], start=False, stop=True)
                            t = spool.tile([128, CKc], f32r, name=f"{tag}_kT{m}",
                                           tag=f"kT{m}", bufs=1)
                            nc.vector.tensor_copy(t[:], ps[:])
                            kT.append(t)
                        # scores for all heads of batch b: (NH, CK)
                        scp = ps_sc.tile([NHc, CKc], f32, name=f"{tag}_scp", tag="sc")
                        for k in range(KTc):
                            nc.tensor.matmul(scp[:], qbd[(b, k)][:], kT[k][:],
                                             start=(k == 0), stop=(k == KTc - 1))
                        E = apool.tile([NHc, CKc], f32, name=f"{tag}_E", tag="E")
                        nc.scalar.activation(E[:], scp[:], AFT.Exp,
                                             accum_out=sums[:, lc:lc + 1])
                        # V chunk (tokens, features), natural
                        vt = []
                        for tj in range(CKc // 128):
                            t = spool.tile([128, Hc], f32r, name=f"{tag}_vt{tj}",
                                           tag=f"vt{tj}", bufs=1)
                            for oc in range(OCH):
                                ps = ps_proj.tile([128, 512], f32, name=f"{tag}_vps",
                                                  tag="proj")
                                for k in range(KTc):
                                    nc.tensor.matmul(
                                        ps[:], kv[k][:, tj * 128:(tj + 1) * 128],
                                        wv_t[k][:, oc * 512:(oc + 1) * 512],
                                        start=(k == 0), stop=False)
                                nc.tensor.matmul(ps[:], ones[:, tj * 128:tj * 128 + 128],
                                                 wv_b[:, oc * 512:(oc + 1) * 512],
                                                 start=False, stop=True)
                                nc.scalar.copy(t[:, oc * 512:(oc + 1) * 512], ps[:])
                            vt.append(t)
                        # E^T chunks and AV accumulation
                        for tj in range(CKc // 128):
                            tp = ps_sc.tile([128, NHc], f32, name=f"{tag}_etp", tag="sc")
                            nc.tensor.transpose(tp[:], E[:, tj * 128:(tj + 1) * 128],
                                                ident[:NHc, :NHc])
                            et = apool.tile([128, NHc], f32r, name=f"{tag}_et{tj}",
                                            tag=f"et{tj}")
                            nc.vector.tensor_copy(et[:], tp[:])
                            for oc in range(OCH):
                                nc.tensor.matmul(
                                    avp[:, oc * 512:(oc + 1) * 512], et[:],
                                    vt[tj][:, oc * 512:(oc + 1) * 512],
                                    start=(lc == 0 and tj == 0),
                                    stop=(lc == nch - 1 and tj == CKc // 128 - 1),
                                    skip_group_check=True)
                    # normalize + transpose to (features, batch-col)
                    tot = apool.tile([NHc, 1], f32, name=f"{tag}_tot{b}", tag="tot")
                    nc.vector.reduce_sum(tot[:], sums[:], axis=AX.X)
                    rec = apool.tile([NHc, 1], f32, name=f"{tag}_rec{b}", tag="rec")
                    nc.vector.reciprocal(rec[:], tot[:])
                    avn = apool.tile([NHc, Hc], f32, name=f"{tag}_avn{b}", tag="avn")
                    nc.vector.tensor_scalar_mul(avn[:], avp[:], rec[:])
                    for j in range(KTc):
                        tp = ps_sc.tile([128, NHc], f32, name=f"{tag}_avtp", tag="sc")
                        nc.tensor.transpose(tp[:], avn[:, j * 128:(j + 1) * 128],
                                            ident[:NHc, :NHc])
                        ha = (j * 128) // HDc
                        hb_ = ((j + 1) * 128 - 1) // HDc
                        # rows of tp are features j*128..j*128+127; head = feat // HD
                        nrow = 128 // HDc  # heads spanned by this 128-feature block
                        for r in range(nrow):
                            hh = ha + r
                            nc.vector.tensor_copy(
                                out_T[j][r * HDc:(r + 1) * HDc, b:b + 1],
                                tp[r * HDc:(r + 1) * HDc, hh:hh + 1])

            # ================= self-attention =================
            with contextlib.ExitStack() as attn_scope:
                apool = attn_scope.enter_context(tc.tile_pool(name="apool", bufs=1))
                spool = attn_scope.enter_context(tc.tile_pool(name="spool", bufs=2))
                wpool = attn_scope.enter_context(tc.tile_pool(name="wpool", bufs=1))

                def load_w(nm, role):
                    wt = []
                    for k in range(KTc):
                        t = wpool.tile([128, Hc], f32r, name=f"w_{nm}{k}",
                                       tag=f"w_{role}{k}")
                        nc.sync.dma_start(t[:], w_attn[nm][k * 128:(k + 1) * 128, :])
                        wt.append(t)
                    bt = wpool.tile([1, Hc], f32r, name=f"wb_{nm}", tag=f"wb_{role}")
                    nc.sync.dma_start(bt[:], w_attn[nm][Hc:Hc + 1, :])
                    return wt, bt

                wq_t, wq_b = load_w("qsa", "q")
                wk_t, wk_b = load_w("ksa", "k")
                wv_t, wv_b = load_w("vsa", "v")
                attention("sa", emb_t, ctx_T, STEPc, wq_t, wq_b, wk_t, wk_b,
                          wv_t, wv_b, sa_T, apool, spool)

                wq_t, wq_b = load_w("qca", "q")
                wk_t, wk_b = load_w("kca", "k")
                wv_t, wv_b = load_w("vca", "v")
                attention("ca", sa_T, enc_T, Sc, wq_t, wq_b, wk_t, wk_b,
                          wv_t, wv_b, ca_T, apool, spool)

            # ================= LSTM cell =================
            with contextlib.ExitStack() as tail_scope:
                lpool = tail_scope.enter_context(tc.tile_pool(name="lpool", bufs=1))
                wstream = tail_scope.enter_context(tc.tile_pool(name="wstream", bufs=6))

                z = emb_t + ca_T + h_t          # 3*KT tiles of (128, BPC)
                ZKT = 3 * KTc
                gates = []
                for g in range(4):
                    t = lpool.tile([BPCc, Hc], f32, name=f"gate{g}", tag=f"gate{g}")
                    gates.append(t)
                for n in range(G4 // 512):
                    ps = ps_proj.tile([BPCc, 512], f32, name="gps", tag="proj")
                    for k in range(ZKT):
                        wt = wstream.tile([128, 512], f32r, name="wcs", tag="wcs")
                        nc.sync.dma_start(
                            wt[:], wc[k * 128:(k + 1) * 128, n * 512:(n + 1) * 512])
                        nc.tensor.matmul(ps[:], z[k][:], wt[:],
                                         start=(k == 0), stop=False)
                    wbt = wstream.tile([1, 512], f32r, name="wcb", tag="wcb")
                    nc.sync.dma_start(wbt[:], wc[ZK:ZK + 1, n * 512:(n + 1) * 512])
                    nc.tensor.matmul(ps[:], ones[:, :BPCc], wbt[:],
                                     start=False, stop=True)
                    g, half = divmod(n, Hc // 512)
                    nc.vector.tensor_copy(
                        gates[g][:, half * 512:(half + 1) * 512], ps[:])
                # elementwise LSTM: i, f, g, o
                i_s = lpool.tile([BPCc, Hc], f32, name="i_s")
                f_s = lpool.tile([BPCc, Hc], f32, name="f_s")
                g_t = lpool.tile([BPCc, Hc], f32, name="g_t")
                o_s = lpool.tile([BPCc, Hc], f32, name="o_s")
                nc.scalar.activation(i_s[:], gates[0][:], AFT.Sigmoid)
                nc.scalar.activation(f_s[:], gates[1][:], AFT.Sigmoid)
                nc.scalar.activation(g_t[:], gates[2][:], AFT.Tanh)
                nc.scalar.activation(o_s[:], gates[3][:], AFT.Sigmoid)
                c_new = lpool.tile([BPCc, Hc], f32, name="c_new")
                tmp = lpool.tile([BPCc, Hc], f32, name="tmp")
                nc.vector.tensor_mul(c_new[:], f_s[:], c_t[:])
                nc.vector.tensor_mul(tmp[:], i_s[:], g_t[:])
                nc.vector.tensor_add(c_new[:], c_new[:], tmp[:])
                tc_new = lpool.tile([BPCc, Hc], f32, name="tc_new")
                nc.scalar.activation(tc_new[:], c_new[:], AFT.Tanh)
                h_new = lpool.tile([BPCc, Hc], f32, name="h_new")
                nc.vector.tensor_mul(h_new[:], o_s[:], tc_new[:])
                nc.sync.dma_start(h_out[:], h_new[:])
                nc.sync.dma_start(c_out[:], c_new[:])

                # ---- transpose h_new -> (H, BPC), AllGather across cores ----
                hb_d = dpool.tile([Hc, BPCc], f32r, name="hb_d")
                for j in range(KTc):
                    tp = ps_sc.tile([128, BPCc], f32, name="htp", tag="sc")
                    nc.tensor.transpose(tp[:], h_new[:, j * 128:(j + 1) * 128],
                                        ident[:BPCc, :BPCc])
                    ht = lpool.tile([128, BPCc], f32r, name="hTn", tag="hTn")
                    nc.vector.tensor_copy(ht[:], tp[:])
                    nc.sync.dma_start(hb_d[j * 128:(j + 1) * 128, :], ht[:])
                hg_d = dpool.tile([NC_N, Hc, BPCc], f32r, name="hg_d")
                nc.gpsimd.collective_compute(
                    "AllGather", mybir.AluOpType.bypass,
                    replica_groups=[list(range(NC_N))],
                    ins=[hb_d[:].opt()], outs=[hg_d[:].opt()])
                h_all = []
                for k in range(KTc):
                    t = lpool.tile([128, B], f32r, name=f"hall{k}", tag=f"hall{k}")
                    nc.sync.dma_start(
                        t[:], hg_d[:, k * 128:(k + 1) * 128, :].rearrange("c f j -> f c j"))
                    h_all.append(t)

                # ---- fc projection over local vocab shard ----
                lg = lpool.tile([B, VSc], f32, name="lg")
                nv = -(-VSc // 512)
                for n in range(nv):
                    cn = min(512, VSc - n * 512)
                    ps = ps_proj.tile([B, 512], f32, name="fps", tag="proj")
                    for k in range(KTc):
                        wt = wstream.tile([128, 512], f32r, name="fcs", tag="fcs")
                        nc.sync.dma_start(
                            wt[:, :cn], fcw[k * 128:(k + 1) * 128, n * 512:n * 512 + cn])
                        nc.tensor.matmul(ps[:, :cn], h_all[k][:], wt[:, :cn],
                                         start=(k == 0), stop=False)
                    wbt = wstream.tile([1, 512], f32r, name="fcb", tag="fcb")
                    nc.sync.dma_start(wbt[:, :cn], fcw[Hc:Hc + 1, n * 512:n * 512 + cn])
                    nc.tensor.matmul(ps[:, :cn], ones[:, :B], wbt[:, :cn],
                                     start=False, stop=True)
                    nc.vector.tensor_copy(lg[:, n * 512:n * 512 + cn], ps[:, :cn])
                nc.sync.dma_start(logits[:], lg[:])

    nc.compile()
    return nc


def _prep_inputs(cfg, inputs):
    Hc, NHc, BPCc = cfg["H"], cfg["NH"], cfg["BPC"]
    Sc, STEPc, VSc = cfg["S"], cfg["STEP"], cfg["VS"]
    HDc = Hc // NHc
    f4 = np.float32

    tok = np.asarray(inputs["target_token"]).reshape(-1).astype(np.int64)
    emb_g = np.asarray(inputs["embedding"], f4)[tok]          # (B, H)
    enc = np.asarray(inputs["encoder_outputs"], f4)           # (B, S, H)
    kv = np.asarray(inputs["kv_cache"], f4)[:, :STEPc, :]     # (B, STEP, H)
    h0 = np.asarray(inputs["decoder_hidden_state"], f4)[0]    # (B, H)
    c0 = np.asarray(inputs["decoder_cell_state"], f4)[0]      # (B, H)

    s = HDc ** -0.5

    def aug(w, b, scale=1.0):
        return np.concatenate([np.asarray(w, f4).T * scale,
                               (np.asarray(b, f4) * scale)[None, :]], axis=0)

    w_qsa = aug(inputs["sa_q_w"], inputs["sa_q_b"], s)
    w_ksa = aug(inputs["sa_k_w"], inputs["sa_k_b"])
    w_vsa = aug(inputs["sa_v_w"], inputs["sa_v_b"])
    w_qca = aug(inputs["ca_q_w"], inputs["ca_q_b"], s)
    w_kca = aug(inputs["ca_k_w"], inputs["ca_k_b"])
    w_vca = aug(inputs["ca_v_w"], inputs["ca_v_b"])

    wih = np.asarray(inputs["lstm_wih"], f4)       # (4H, 2H)
    whh = np.asarray(inputs["lstm_whh"], f4)       # (4H, H)
    bsum = (np.asarray(inputs["lstm_bih"], f4) + np.asarray(inputs["lstm_bhh"], f4))
    wc = np.concatenate([wih.T, whh.T, bsum[None, :]], axis=0)  # (3H+1, 4H)

    fc_w = np.asarray(inputs["fc_w"], f4)          # (V, H)
    fc_b = np.asarray(inputs["fc_b"], f4)
    Vc = fc_w.shape[0]
    fcw_full = np.zeros((Hc + 1, VSc * NC_N), f4)
    fcw_full[:Hc, :Vc] = fc_w.T
    fcw_full[Hc, :Vc] = fc_b

    qmask = np.zeros((Hc, NHc), f4)
    for f in range(Hc):
        qmask[f, f // HDc] = 1.0
    ones_in = np.ones((1, 512), f4)

    in_maps = []
    for c in range(NC_N):
        sl = slice(c * BPCc, (c + 1) * BPCc)
        in_maps.append({
            "enc_T": np.ascontiguousarray(enc[sl].transpose(0, 2, 1)),
            "ctx_T": np.ascontiguousarray(kv[sl].transpose(0, 2, 1)),
            "emb_T": np.ascontiguousarray(emb_g[sl].T),
            "h_T": np.ascontiguousarray(h0[sl].T),
            "c_nat": np.ascontiguousarray(c0[sl]),
            "w_qsa": w_qsa, "w_ksa": w_ksa, "w_vsa": w_vsa,
            "w_qca": w_qca, "w_kca": w_kca, "w_vca": w_vca,
            "wc": wc,
            "fcw": np.ascontiguousarray(fcw_full[:, c * VSc:(c + 1) * VSc]),
            "qmask": qmask,
            "ones_in": ones_in,
        })
    return in_maps


def _postprocess(results):
    logits = np.concatenate([r["logits"] for r in results], axis=1)[:, :V]
    h_new = np.concatenate([results[c]["h_out"] for c in range(NC_N)], axis=0)
    c_new = np.concatenate([results[c]["c_out"] for c in range(NC_N)], axis=0)
    return logits, h_new[None], c_new[None]


def kernel(**inputs):
    from concourse import bass_utils
    step = int(np.asarray(inputs["decoder_step"]))
    cfg = _full_cfg(step)
    key = tuple(sorted(cfg.items()))
    if key not in _BUILD_CACHE:
        _BUILD_CACHE[key] = _build(cfg)
    nc = _BUILD_CACHE[key]
    in_maps = _prep_inputs(cfg, inputs)
    res = bass_utils.run_bass_kernel_spmd(nc, in_maps, core_ids=list(range(NC_N)))
    return _postprocess(res.results)


# revision 6
# speedup vs baseline: 1.2050x; 1.2050x over previous
"""Trainium2 Bass kernel for nn_DecoderSACA (single-step decoder w/ self+cross
attention, LSTM cell, and vocab projection).

Sharding: data-parallel over batch B=64 across 8 NeuronCores (8 batches/core)
for everything through the LSTM; the final fc projection is tensor-parallel
over the (padded) vocab after an on-device AllGather of h_new.

Device layout convention: activations are stored transposed (feature on the
SBUF partition dim, batch/token on the free dim) so every matmul contracts
along partitions naturally.  All biases are folded in with a ones-row trick.
Attention scores for all 16 heads of one batch come from a single matmul
chain with a block-diagonal Q operand (built on device with a constant mask).
Matmul inputs use the float32r dtype (TF32-like, full PE rate at N=512).
"""

import numpy as np
import ml_dtypes
BF16 = ml_dtypes.bfloat16

# ---- problem constants ----
B, S, H, NH, V = 64, 2048, 1024, 16, 50257
MAXLEN = 1024
HD = H // NH           # 64
NC_N = 8               # cores
BPC = B // NC_N        # 8 batches per core
CK = 512               # token chunk size
KT = H // 128          # 8 feature ktiles
# vocab shard: pad V so shards are equal and even (fp32r matmuls need an
# even moving-operand innermost count); each core handles VS columns
VS = (V + NC_N - 1) // NC_N
VS += VS % 2                            # 6284
VPAD = VS * NC_N                        # 50272


def _full_cfg(step):
    return dict(S=S, STEP=int(step), VS=VS, H=H, NH=NH, BPC=BPC, CK=CK)


_BUILD_CACHE = {}


def _build(cfg):
    import concourse.bass as bass
    import concourse.tile as tile
    from concourse import bacc, mybir, masks

    f32, f32r = mybir.dt.float32, mybir.dt.float32r
    bf16 = mybir.dt.bfloat16
    AFT = mybir.ActivationFunctionType
    AX = mybir.AxisListType

    Hc, NHc, BPCc, CKc = cfg["H"], cfg["NH"], cfg["BPC"], cfg["CK"]
    Sc, STEPc, VSc = cfg["S"], cfg["STEP"], cfg["VS"]
    KTc = Hc // 128
    HDc = Hc // NHc
    G4 = 4 * Hc                       # gates width
    ZK = 3 * Hc                       # z contraction (emb + ca + h), + ones row
    assert CKc % 128 == 0 and Hc % 512 == 0 and Sc % CKc == 0 and STEPc % CKc == 0

    nc = bacc.Bacc("TRN2", target_bir_lowering=False, debug=False, num_devices=NC_N)

    def din(name, shape, dt=f32r):
        return nc.dram_tensor(name, list(shape), dt, kind="ExternalInput").ap()

    def dout(name, shape, dt=f32):
        return nc.dram_tensor(name, list(shape), dt, kind="ExternalOutput").ap()

    enc_T = din("enc_T", (BPCc, Hc, Sc), bf16)
    ctx_T = din("ctx_T", (BPCc, Hc, STEPc), bf16)
    emb_T = din("emb_T", (Hc, BPCc))
    h_T = din("h_T", (Hc, BPCc))
    c_nat = din("c_nat", (BPCc, Hc), f32)
    w_attn = {}
    for nm in ("qsa", "qca"):
        w_attn[nm] = din(f"w_{nm}", (Hc + 1, Hc))
    for nm in ("ksa", "vsa", "kca", "vca"):
        w_attn[nm] = din(f"w_{nm}", (Hc + 1, Hc), bf16)
    wc = din("wc", (ZK + 1, G4))
    fcw = din("fcw", (Hc + 1, VSc))
    qmask = din("qmask", (Hc, NHc), f32)
    ones_in = din("ones_in", (1, 512))
    onesb_in = din("onesb_in", (1, 512), bf16)

    logits = dout("logits", (B, VSc))
    h_out = dout("h_out", (BPCc, Hc))
    c_out = dout("c_out", (BPCc, Hc))

    OCH = Hc // 512                   # output-feature 512-chunks (2)

    with tile.TileContext(nc) as tc:
        import contextlib
        est = contextlib.ExitStack()
        with est:
            persist = est.enter_context(tc.tile_pool(name="persist", bufs=1))
            ps_proj = est.enter_context(tc.tile_pool(name="ps_proj", bufs=4, space="PSUM"))
            ps_sc = est.enter_context(tc.tile_pool(name="ps_sc", bufs=2, space="PSUM"))
            ps_av = est.enter_context(tc.tile_pool(name="ps_av", bufs=1, space="PSUM"))
            dpool = est.enter_context(tc.tile_pool(name="dram", bufs=1, space="DRAM"))

            ident = persist.tile([128, 128], f32, name="ident")
            masks.make_identity(nc, ident[:])
            ones = persist.tile([1, 512], f32r, name="ones")
            nc.sync.dma_start(ones[:], ones_in[:])
            onesb = persist.tile([1, 512], bf16, name="onesb")
            nc.sync.dma_start(onesb[:], onesb_in[:])
            qm = []
            for k in range(KTc):
                t = persist.tile([128, NHc], f32, name=f"qm{k}", tag=f"qm{k}")
                nc.sync.dma_start(t[:], qmask[k * 128:(k + 1) * 128, :])
                qm.append(t)
            emb_t, h_t = [], []
            for k in range(KTc):
                t = persist.tile([128, BPCc], f32r, name=f"emb{k}", tag=f"emb{k}")
                nc.sync.dma_start(t[:], emb_T[k * 128:(k + 1) * 128, :])
                emb_t.append(t)
                t = persist.tile([128, BPCc], f32r, name=f"ht{k}", tag=f"ht{k}")
                nc.sync.dma_start(t[:], h_T[k * 128:(k + 1) * 128, :])
                h_t.append(t)
            c_t = persist.tile([BPCc, Hc], f32, name="c_t")
            nc.sync.dma_start(c_t[:], c_nat[:])
            sa_T = [persist.tile([128, BPCc], f32r, name=f"saT{k}", tag=f"saT{k}")
                    for k in range(KTc)]
            ca_T = [persist.tile([128, BPCc], f32r, name=f"caT{k}", tag=f"caT{k}")
                    for k in range(KTc)]

            def attention(tag, q_src, kv_dram, L, wq_t, wq_b, wk_t, wk_b, wv_t, wv_b,
                          out_T, apool, spool):
                """q_src: KT sbuf tiles (128, BPC) f32r.  kv_dram: (BPC, H, L) dram.
                Writes out_T: KT tiles (128, BPC) f32r (features x batch)."""
                nch = L // CKc
                # ---- Q projection + block-diagonal Q ----
                qT = []
                for m in range(KTc):
                    ps = ps_sc.tile([128, BPCc], f32, name=f"{tag}_qps{m}", tag="sc")
                    for k in range(KTc):
                        nc.tensor.matmul(ps[:], wq_t[k][:, m * 128:(m + 1) * 128],
                                         q_src[k][:], start=(k == 0), stop=False)
                    nc.tensor.matmul(ps[:], wq_b[:, m * 128:(m + 1) * 128],
                                     ones[:, :BPCc], start=False, stop=True)
                    t = apool.tile([128, BPCc], f32, name=f"{tag}_qT{m}", tag=f"{tag}_qT{m}")
                    nc.vector.tensor_copy(t[:], ps[:])
                    qT.append(t)
                qbd = {}
                for b in range(BPCc):
                    for k in range(KTc):
                        t = apool.tile([128, NHc], f32r, name=f"{tag}_qbd{b}_{k}",
                                       tag=f"{tag}_qbd{b}_{k}")
                        nc.vector.tensor_scalar_mul(t[:], qm[k][:], qT[k][:, b:b + 1])
                        qbd[(b, k)] = t
                # ---- chunked K/V projection + attention ----
                for b in range(BPCc):
                    sums = apool.tile([NHc, nch], f32, name=f"{tag}_sums{b}", tag="sums")
                    avp = ps_av.tile([NHc, Hc], f32, name=f"{tag}_avp{b}", tag="av")
                    for lc in range(nch):
                        col0 = lc * CKc
                        kv = []
                        for k in range(KTc):
                            t = spool.tile([128, CKc], bf16, name=f"{tag}_kv{k}",
                                           tag=f"kv{k}")
                            nc.sync.dma_start(
                                t[:], kv_dram[b, k * 128:(k + 1) * 128, col0:col0 + CKc])
                            kv.append(t)
                        # K^T chunk: (features, CK)
                        kT = []
                        for m in range(KTc):
                            ps = ps_proj.tile([128, CKc], f32, name=f"{tag}_kps{m}",
                                              tag="proj")
                            for k in range(KTc):
                                nc.tensor.matmul(ps[:], wk_t[k][:, m * 128:(m + 1) * 128],
                                                 kv[k][:], start=(k == 0), stop=False)
                            nc.tensor.matmul(ps[:], wk_b[:, m * 128:(m + 1) * 128],
                                             onesb[:, :CKc], start=False, stop=True)
                            t = spool.tile([128, CKc], f32r, name=f"{tag}_kT{m}",
                                           tag=f"kT{m}", bufs=1)
                            nc.vector.tensor_copy(t[:], ps[:])
                            kT.append(t)
                        # scores for all heads of batch b: (NH, CK)
                        scp = ps_sc.tile([NHc, CKc], f32, name=f"{tag}_scp", tag="sc")
                        for k in range(KTc):
                            nc.tensor.matmul(scp[:], qbd[(b, k)][:], kT[k][:],
                                             start=(k == 0), stop=(k == KTc - 1))
                        E = apool.tile([NHc, CKc], f32, name=f"{tag}_E", tag="E")
                        nc.scalar.activation(E[:], scp[:], AFT.Exp,
                                             accum_out=sums[:, lc:lc + 1])
                        # V chunk (tokens, features), natural
                        vt = []
                        for tj in range(CKc // 128):
                            t = spool.tile([128, Hc], f32r, name=f"{tag}_vt{tj}",
                                           tag=f"vt{tj}", bufs=1)
                            for oc in range(OCH):
                                ps = ps_proj.tile([128, 512], f32, name=f"{tag}_vps",
                                                  tag="proj")
                                for k in range(KTc):
                                    nc.tensor.matmul(
                                        ps[:], kv[k][:, tj * 128:(tj + 1) * 128],
                                        wv_t[k][:, oc * 512:(oc + 1) * 512],
                                        start=(k == 0), stop=False)
                                nc.tensor.matmul(ps[:], onesb[:, tj * 128:tj * 128 + 128],
                                                 wv_b[:, oc * 512:(oc + 1) * 512],
                                                 start=False, stop=True)
                                nc.scalar.copy(t[:, oc * 512:(oc + 1) * 512], ps[:])
                            vt.append(t)
                        # E^T chunks and AV accumulation
                        for tj in range(CKc // 128):
                            tp = ps_sc.tile([128, NHc], f32, name=f"{tag}_etp", tag="sc")
                            nc.tensor.transpose(tp[:], E[:, tj * 128:(tj + 1) * 128],
                                                ident[:NHc, :NHc])
                            et = apool.tile([128, NHc], f32r, name=f"{tag}_et{tj}",
                                            tag=f"et{tj}")
                            nc.vector.tensor_copy(et[:], tp[:])
                            for oc in range(OCH):
                                nc.tensor.matmul(
                                    avp[:, oc * 512:(oc + 1) * 512], et[:],
                                    vt[tj][:, oc * 512:(oc + 1) * 512],
                                    start=(lc == 0 and tj == 0),
                                    stop=(lc == nch - 1 and tj == CKc // 128 - 1),
                                    skip_group_check=True)
                    # normalize + transpose to (features, batch-col)
                    tot = apool.tile([NHc, 1], f32, name=f"{tag}_tot{b}", tag="tot")
                    nc.vector.reduce_sum(tot[:], sums[:], axis=AX.X)
                    rec = apool.tile([NHc, 1], f32, name=f"{tag}_rec{b}", tag="rec")
                    nc.vector.reciprocal(rec[:], tot[:])
                    avn = apool.tile([NHc, Hc], f32, name=f"{tag}_avn{b}", tag="avn")
                    nc.vector.tensor_scalar_mul(avn[:], avp[:], rec[:])
                    for j in range(KTc):
                        tp = ps_sc.tile([128, NHc], f32, name=f"{tag}_avtp", tag="sc")
                        nc.tensor.transpose(tp[:], avn[:, j * 128:(j + 1) * 128],
                                            ident[:NHc, :NHc])
                        ha = (j * 128) // HDc
                        hb_ = ((j + 1) * 128 - 1) // HDc
                        # rows of tp are features j*128..j*128+127; head = feat // HD
                        nrow = 128 // HDc  # heads spanned by this 128-feature block
                        for r in range(nrow):
                            hh = ha + r
                            nc.vector.tensor_copy(
                                out_T[j][r * HDc:(r + 1) * HDc, b:b + 1],
                                tp[r * HDc:(r + 1) * HDc, hh:hh + 1])

            # ================= self-attention =================
            with contextlib.ExitStack() as attn_scope:
                apool = attn_scope.enter_context(tc.tile_pool(name="apool", bufs=1))
                spool = attn_scope.enter_context(tc.tile_pool(name="spool", bufs=2))
                wpool = attn_scope.enter_context(tc.tile_pool(name="wpool", bufs=1))

                def load_w(nm, role):
                    dt_w = f32r if role == "q" else bf16
                    wt = []
                    for k in range(KTc):
                        t = wpool.tile([128, Hc], dt_w, name=f"w_{nm}{k}",
                                       tag=f"w_{role}{k}")
                        nc.sync.dma_start(t[:], w_attn[nm][k * 128:(k + 1) * 128, :])
                        wt.append(t)
                    bt = wpool.tile([1, Hc], dt_w, name=f"wb_{nm}", tag=f"wb_{role}")
                    nc.sync.dma_start(bt[:], w_attn[nm][Hc:Hc + 1, :])
                    return wt, bt

                wq_t, wq_b = load_w("qsa", "q")
                wk_t, wk_b = load_w("ksa", "k")
                wv_t, wv_b = load_w("vsa", "v")
                attention("sa", emb_t, ctx_T, STEPc, wq_t, wq_b, wk_t, wk_b,
                          wv_t, wv_b, sa_T, apool, spool)

                wq_t, wq_b = load_w("qca", "q")
                wk_t, wk_b = load_w("kca", "k")
                wv_t, wv_b = load_w("vca", "v")
                attention("ca", sa_T, enc_T, Sc, wq_t, wq_b, wk_t, wk_b,
                          wv_t, wv_b, ca_T, apool, spool)

            # ================= LSTM cell =================
            with contextlib.ExitStack() as tail_scope:
                lpool = tail_scope.enter_context(tc.tile_pool(name="lpool", bufs=1))
                wstream = tail_scope.enter_context(tc.tile_pool(name="wstream", bufs=6))

                z = emb_t + ca_T + h_t          # 3*KT tiles of (128, BPC)
                ZKT = 3 * KTc
                gates = []
                for g in range(4):
                    t = lpool.tile([BPCc, Hc], f32, name=f"gate{g}", tag=f"gate{g}")
                    gates.append(t)
                for n in range(G4 // 512):
                    ps = ps_proj.tile([BPCc, 512], f32, name="gps", tag="proj")
                    for k in range(ZKT):
                        wt = wstream.tile([128, 512], f32r, name="wcs", tag="wcs")
                        nc.sync.dma_start(
                            wt[:], wc[k * 128:(k + 1) * 128, n * 512:(n + 1) * 512])
                        nc.tensor.matmul(ps[:], z[k][:], wt[:],
                                         start=(k == 0), stop=False)
                    wbt = wstream.tile([1, 512], f32r, name="wcb", tag="wcb")
                    nc.sync.dma_start(wbt[:], wc[ZK:ZK + 1, n * 512:(n + 1) * 512])
                    nc.tensor.matmul(ps[:], ones[:, :BPCc], wbt[:],
                                     start=False, stop=True)
                    g, half = divmod(n, Hc // 512)
                    nc.vector.tensor_copy(
                        gates[g][:, half * 512:(half + 1) * 512], ps[:])
                # elementwise LSTM: i, f, g, o
                i_s = lpool.tile([BPCc, Hc], f32, name="i_s")
                f_s = lpool.tile([BPCc, Hc], f32, name="f_s")
                g_t = lpool.tile([BPCc, Hc], f32, name="g_t")
                o_s = lpool.tile([BPCc, Hc], f32, name="o_s")
                nc.scalar.activation(i_s[:], gates[0][:], AFT.Sigmoid)
                nc.scalar.activation(f_s[:], gates[1][:], AFT.Sigmoid)
                nc.scalar.activation(g_t[:], gates[2][:], AFT.Tanh)
                nc.scalar.activation(o_s[:], gates[3][:], AFT.Sigmoid)
                c_new = lpool.tile([BPCc, Hc], f32, name="c_new")
                tmp = lpool.tile([BPCc, Hc], f32, name="tmp")
                nc.vector.tensor_mul(c_new[:], f_s[:], c_t[:])
                nc.vector.tensor_mul(tmp[:], i_s[:], g_t[:])
                nc.vector.tensor_add(c_new[:], c_new[:], tmp[:])
                tc_new = lpool.tile([BPCc, Hc], f32, name="tc_new")
                nc.scalar.activation(tc_new[:], c_new[:], AFT.Tanh)
                h_new = lpool.tile([BPCc, Hc], f32, name="h_new")
                nc.vector.tensor_mul(h_new[:], o_s[:], tc_new[:])
                nc.sync.dma_start(h_out[:], h_new[:])
                nc.sync.dma_start(c_out[:], c_new[:])

                # ---- transpose h_new -> (H, BPC), AllGather across cores ----
                hb_d = dpool.tile([Hc, BPCc], f32r, name="hb_d")
                for j in range(KTc):
                    tp = ps_sc.tile([128, BPCc], f32, name="htp", tag="sc")
                    nc.tensor.transpose(tp[:], h_new[:, j * 128:(j + 1) * 128],
                                        ident[:BPCc, :BPCc])
                    ht = lpool.tile([128, BPCc], f32r, name="hTn", tag="hTn")
                    nc.vector.tensor_copy(ht[:], tp[:])
                    nc.sync.dma_start(hb_d[j * 128:(j + 1) * 128, :], ht[:])
                hg_d = dpool.tile([NC_N, Hc, BPCc], f32r, name="hg_d")
                nc.gpsimd.collective_compute(
                    "AllGather", mybir.AluOpType.bypass,
                    replica_groups=[list(range(NC_N))],
                    ins=[hb_d[:].opt()], outs=[hg_d[:].opt()])
                h_all = []
                for k in range(KTc):
                    t = lpool.tile([128, B], f32r, name=f"hall{k}", tag=f"hall{k}")
                    nc.sync.dma_start(
                        t[:], hg_d[:, k * 128:(k + 1) * 128, :].rearrange("c f j -> f c j"))
                    h_all.append(t)

                # ---- fc projection over local vocab shard ----
                lg = lpool.tile([B, VSc], f32, name="lg")
                nv = -(-VSc // 512)
                for n in range(nv):
                    cn = min(512, VSc - n * 512)
                    ps = ps_proj.tile([B, 512], f32, name="fps", tag="proj")
                    for k in range(KTc):
                        wt = wstream.tile([128, 512], f32r, name="fcs", tag="fcs")
                        nc.sync.dma_start(
                            wt[:, :cn], fcw[k * 128:(k + 1) * 128, n * 512:n * 512 + cn])
                        nc.tensor.matmul(ps[:, :cn], h_all[k][:], wt[:, :cn],
                                         start=(k == 0), stop=False)
                    wbt = wstream.tile([1, 512], f32r, name="fcb", tag="fcb")
                    nc.sync.dma_start(wbt[:, :cn], fcw[Hc:Hc + 1, n * 512:n * 512 + cn])
                    nc.tensor.matmul(ps[:, :cn], ones[:, :B], wbt[:, :cn],
                                     start=False, stop=True)
                    nc.vector.tensor_copy(lg[:, n * 512:n * 512 + cn], ps[:, :cn])
                nc.sync.dma_start(logits[:], lg[:])

    nc.compile()
    return nc


def _prep_inputs(cfg, inputs):
    Hc, NHc, BPCc = cfg["H"], cfg["NH"], cfg["BPC"]
    Sc, STEPc, VSc = cfg["S"], cfg["STEP"], cfg["VS"]
    HDc = Hc // NHc
    f4 = np.float32

    tok = np.asarray(inputs["target_token"]).reshape(-1).astype(np.int64)
    emb_g = np.asarray(inputs["embedding"], f4)[tok]          # (B, H)
    enc = np.asarray(inputs["encoder_outputs"], f4)           # (B, S, H)
    kv = np.asarray(inputs["kv_cache"], f4)[:, :STEPc, :]     # (B, STEP, H)
    h0 = np.asarray(inputs["decoder_hidden_state"], f4)[0]    # (B, H)
    c0 = np.asarray(inputs["decoder_cell_state"], f4)[0]      # (B, H)

    s = HDc ** -0.5

    def aug(w, b, scale=1.0):
        return np.concatenate([np.asarray(w, f4).T * scale,
                               (np.asarray(b, f4) * scale)[None, :]], axis=0)

    w_qsa = aug(inputs["sa_q_w"], inputs["sa_q_b"], s)
    w_ksa = aug(inputs["sa_k_w"], inputs["sa_k_b"])
    w_vsa = aug(inputs["sa_v_w"], inputs["sa_v_b"])
    w_qca = aug(inputs["ca_q_w"], inputs["ca_q_b"], s)
    w_kca = aug(inputs["ca_k_w"], inputs["ca_k_b"])
    w_vca = aug(inputs["ca_v_w"], inputs["ca_v_b"])

    wih = np.asarray(inputs["lstm_wih"], f4)       # (4H, 2H)
    whh = np.asarray(inputs["lstm_whh"], f4)       # (4H, H)
    bsum = (np.asarray(inputs["lstm_bih"], f4) + np.asarray(inputs["lstm_bhh"], f4))
    wc = np.concatenate([wih.T, whh.T, bsum[None, :]], axis=0)  # (3H+1, 4H)

    fc_w = np.asarray(inputs["fc_w"], f4)          # (V, H)
    fc_b = np.asarray(inputs["fc_b"], f4)
    Vc = fc_w.shape[0]
    fcw_full = np.zeros((Hc + 1, VSc * NC_N), f4)
    fcw_full[:Hc, :Vc] = fc_w.T
    fcw_full[Hc, :Vc] = fc_b

    qmask = np.zeros((Hc, NHc), f4)
    for f in range(Hc):
        qmask[f, f // HDc] = 1.0
    ones_in = np.ones((1, 512), f4)

    in_maps = []
    for c in range(NC_N):
        sl = slice(c * BPCc, (c + 1) * BPCc)
        in_maps.append({
            "enc_T": np.ascontiguousarray(enc[sl].transpose(0, 2, 1)).astype(BF16),
            "ctx_T": np.ascontiguousarray(kv[sl].transpose(0, 2, 1)).astype(BF16),
            "emb_T": np.ascontiguousarray(emb_g[sl].T),
            "h_T": np.ascontiguousarray(h0[sl].T),
            "c_nat": np.ascontiguousarray(c0[sl]),
            "w_qsa": w_qsa, "w_ksa": w_ksa.astype(BF16), "w_vsa": w_vsa.astype(BF16),
            "w_qca": w_qca, "w_kca": w_kca.astype(BF16), "w_vca": w_vca.astype(BF16),
            "wc": wc,
            "fcw": np.ascontiguousarray(fcw_full[:, c * VSc:(c + 1) * VSc]),
            "qmask": qmask,
            "ones_in": ones_in, "onesb_in": ones_in.astype(BF16),
        })
    return in_maps


def _postprocess(results):
    logits = np.concatenate([r["logits"] for r in results], axis=1)[:, :V]
    h_new = np.concatenate([results[c]["h_out"] for c in range(NC_N)], axis=0)
    c_new = np.concatenate([results[c]["c_out"] for c in range(NC_N)], axis=0)
    return logits, h_new[None], c_new[None]


def kernel(**inputs):
    from concourse import bass_utils
    step = int(np.asarray(inputs["decoder_step"]))
    cfg = _full_cfg(step)
    key = tuple(sorted(cfg.items()))
    if key not in _BUILD_CACHE:
        _BUILD_CACHE[key] = _build(cfg)
    nc = _BUILD_CACHE[key]
    in_maps = _prep_inputs(cfg, inputs)
    res = bass_utils.run_bass_kernel_spmd(nc, in_maps, core_ids=list(range(NC_N)))
    return _postprocess(res.results)
